# revision 24
# baseline (speedup 1.0000x reference)
"""Trainium2 Bass kernel for the water-network leak MSE model.

Math (reference):
    net(s)   = base[idx_s] + MLP(idx_s)                    (idx_s in [0,1024))
    y        = net*onehot(idx) @ M^T + demand              demand[:, 2j] = D[:, j]
    q        = y @ inv
    hL       = sign(q) * K * |q|^1.852,  K = 10.667 C^-1.852 d^-4.871 L
    H        = (supply - hL) @ inv^T
    d_leak   = Cd*a*sqrt(2g) * (onehot @ M^T) * sqrt(relu(H))
    out      = mean((q @ A0^T - demand - d_leak)^2)

Device strategy (8 cores, data-parallel over samples, 2048 samples/core):
  Host folds (validated against the reference, adds ~1e-3 rel err total,
  tolerance is 2e-2):
    - PM = inv^T M, AM = A0' PM with the memoized per-pipe net table and the
      Hazen-Williams coefficient k1 = K^(1/1.852) premultiplied; node rows
      permuted even-first; demand folded into the A0inv stationary (-I).
    - leak_id is host-visible, so the per-sample M/PM/AM columns are
      gathered ON THE HOST into dense [feature, sample] inputs.
    - All matmul stationaries and the M/AM gather payloads ship as fp8-e4m3
      (PE runs them mixed against bf16 movings); the PM payload and D stay
      bf16. invpt carries a per-pipe power-of-2 scale s (calibrated from a
      host sample of hl's rms) to center fp8 range; s is folded out of hl
      via k1 so the product is exact.
    - Inputs stream in consumption order through a single DMA queue so the
      startup is HBM-bound, not issue-bound.
  The two power chains use exponent bit-hacks on bf16 bit patterns (no ACT
  tables at all):
    |q|^0.852: e_bits = round(0.852*(q_bits & 0x7fff)) + 2406, hl = q*e
    c0*sqrt(relu(H)): rl = Relu(-hp + c0^2*hsup) on ACT (c0^2 folded into
                      invpt/hsup), then s_bits = round(0.5*rl_bits) + 8124
  Per 512-sample chunk: PE does q (K=256) and H (K=1024) matmuls plus
  identity-injects of (AM_col - d_leak) into the residual PSUM; DVE drains
  q-PSUM in 1024-wide bank pairs (+PM_col), does the bit-hacks and d_leak
  elementwise; ACT does the 0.852 fma (int16 Identity), Relu, and the final
  Square+accumulate from PSUM. The residual stage of chunk sc is emitted
  during chunk sc+1 so the PE stream never stalls on the q->hl chain.
  Each core returns [128, 16] partial sums of squares; host reduces.
"""

import math

import numpy as np
import ml_dtypes

P = 128
N_CORES = 8
S_TOTAL = 16384
SC = S_TOTAL // N_CORES  # samples per core
CH = 512                 # samples per chunk
NCH = SC // CH           # chunks per core
N_NODES = 512
N_PIPES = 1024
N_DEM = 256
G_ACC = 9.80665

CP = 2406.0   # pow-hack bias:  e_bits = round(0.852*t) + CP
CS = 8124.0   # sqrt-hack bias: s_bits = round(0.5*t) + CS

BF16 = ml_dtypes.bfloat16
F8 = ml_dtypes.float8_e4m3

# bf16 stream layout (per chunk: dt chunk-major, PM gather block, M|AM block)
BCH = CH * 2 + 8 * CH + 8 * CH  # 9216 cols per chunk
B_DT = 0                        # dt offset within a chunk slice
B_PM = 2 * CH                   # gPM offset within a chunk slice
B_MA = 10 * CH                  # gM|gAM offset within a chunk slice
# fp8 stream layout (matmul stationaries only)
W8_INVEV = 0                    # 16 blocks * 128
W8_INVPT = 16 * P               # 32 blocks * 128
W8_A0INV = 48 * P               # 8 blocks * 128
W8_END = 56 * P

_MODULE_CACHE: dict = {}


def _build_module():
    import concourse.bacc as bacc
    import concourse.mybir as mybir
    import concourse.tile as tile

    f32 = mybir.dt.float32
    bf16 = mybir.dt.bfloat16
    i16 = mybir.dt.int16
    fp8 = mybir.dt.float8e4
    AF = mybir.ActivationFunctionType
    OP = mybir.AluOpType

    nc = bacc.Bacc(trn_type="TRN2", target_bir_lowering=False, debug=False)

    consts_d = nc.dram_tensor("consts", [P, 70], f32, kind="ExternalInput").ap()
    big8_d = nc.dram_tensor("big8", [P, W8_END], fp8, kind="ExternalInput").ap()
    bigb_d = nc.dram_tensor("bigb", [P, NCH * BCH], bf16, kind="ExternalInput").ap()
    out_d = nc.dram_tensor("out_stats", [P, NCH * 4], f32, kind="ExternalOutput").ap()

    with tile.TileContext(nc) as tc:
        with (
            tc.tile_pool(name="const", bufs=1) as cpool,
            tc.tile_pool(name="work", bufs=1) as wpool,
            tc.tile_pool(name="small", bufs=2) as spool,
            tc.tile_pool(name="qps", bufs=2, space="PSUM") as qpool,
            tc.tile_pool(name="hps", bufs=2, space="PSUM") as hpool,
            tc.tile_pool(name="rps", bufs=2, space="PSUM") as rpool,
        ):
            # ---- input stream, in consumption order on one queue ----
            consts = cpool.tile_from(consts_d)
            biases = consts[:, 0:2]
            hsup = consts[:, 2:6]
            ident = consts.bitcast(bf16)[:, 12:140]

            invev = cpool.tile([P, 16 * P], fp8, tag="invev")
            nc.sync.dma_start(invev, big8_d[:, W8_INVEV : W8_INVEV + 16 * P])
            btiles = []
            for sc in range(NCH):
                btiles.append(
                    cpool.tile([P, BCH], bf16, tag=f"bt{sc}", name=f"bt{sc}")
                )
            nc.sync.dma_start(btiles[0][:, 0:B_MA], bigb_d[:, 0:B_MA])
            wt8 = cpool.tile([P, 40 * P], fp8, tag="wt8")  # invpt | a0inv
            nc.sync.dma_start(wt8, big8_d[:, W8_INVPT:W8_END])
            invpt = wt8[:, 0 : 32 * P]
            a0inv = wt8[:, 32 * P : 40 * P]
            nc.sync.dma_start(btiles[0][:, B_MA:BCH], bigb_d[:, B_MA:BCH])
            for sc in range(1, NCH):
                nc.sync.dma_start(
                    btiles[sc][:, 0:B_MA],
                    bigb_d[:, sc * BCH : sc * BCH + B_MA],
                )
                nc.sync.dma_start(
                    btiles[sc][:, B_MA:BCH],
                    bigb_d[:, sc * BCH + B_MA : (sc + 1) * BCH],
                )
            stats = cpool.tile([P, NCH * 4], f32, tag="stats")

            # residual stage of chunk sc runs during chunk sc+1: the PE
            # matmuls right after chunk sc+1's early q pairs, the ACT
            # squares after chunk sc+1's fma chain
            pending_resid = []
            pending_square = []

            def emit_resid_d(state):
                sc, bt, amdls = state
                rps = []
                for n_ in range(4):
                    rp = rpool.tile([P, CH], f32, tag="rp")
                    nc.tensor.matmul(
                        rp,
                        a0inv[:, (0 * 4 + n_) * P : (0 * 4 + n_ + 1) * P],
                        bt[:, B_DT : B_DT + CH],
                        start=True,
                        stop=False,
                    )
                    nc.tensor.matmul(
                        rp,
                        a0inv[:, (1 * 4 + n_) * P : (1 * 4 + n_ + 1) * P],
                        bt[:, B_DT + CH : B_DT + 2 * CH],
                        start=False,
                        stop=False,
                    )
                    rps.append(rp)
                return rps

            def emit_resid_inject(state, rps):
                sc, bt, amdls = state
                for n_ in range(4):
                    nc.tensor.matmul(rps[n_], ident, amdls[n_], start=False, stop=True)
                pending_square.append((sc, rps))

            def emit_squares():
                sc, rps = pending_square.pop()
                scr = spool.tile([P, CH], bf16, tag="scr")
                for n_ in range(4):
                    nc.scalar.activation(
                        scr,
                        rps[n_],
                        AF.Square,
                        accum_out=stats[:, sc * 4 + n_ : sc * 4 + n_ + 1],
                    )

            def make_ctx(sc):
                ctx = {"sc": sc, "bt": btiles[sc], "qsbp": [], "absqp": [],
                       "ebp": [], "hlp": []}
                for pj in range(4):
                    ctx["qsbp"].append(wpool.tile(
                        [P, 2 * CH], bf16, tag=f"qsbp{pj}",
                        name=f"qsbp{pj}_{sc}", bufs=2))
                    ctx["absqp"].append(wpool.tile(
                        [P, 2 * CH], i16, tag=f"absqp{pj}",
                        name=f"absqp{pj}_{sc}", bufs=2))
                    ctx["ebp"].append(wpool.tile(
                        [P, 2 * CH], i16, tag=f"ebp{pj}",
                        name=f"ebp{pj}_{sc}", bufs=2))
                    ctx["hlp"].append(wpool.tile(
                        [P, 2 * CH], bf16, tag=f"hlp{pj}",
                        name=f"hlp{pj}_{sc}", bufs=2))
                return ctx

            def q_pair(ctx, pj):
                bt = ctx["bt"]
                qp2 = qpool.tile([P, 2 * CH], f32, tag="qp2")
                for h in range(2):
                    pc = pj * 2 + h
                    dst = qp2[:, h * CH : (h + 1) * CH]
                    nc.tensor.matmul(
                        dst,
                        invev[:, (0 * 8 + pc) * P : (0 * 8 + pc + 1) * P],
                        bt[:, B_DT : B_DT + CH],
                        start=True,
                        stop=False,
                    )
                    nc.tensor.matmul(
                        dst,
                        invev[:, (1 * 8 + pc) * P : (1 * 8 + pc + 1) * P],
                        bt[:, B_DT + CH : B_DT + 2 * CH],
                        start=False,
                        stop=True,
                    )
                nc.vector.tensor_tensor(
                    ctx["qsbp"][pj],
                    qp2,
                    bt[:, B_PM + pj * 2 * CH : B_PM + (pj + 1) * 2 * CH],
                    OP.add,
                )
                nc.vector.tensor_scalar(
                    ctx["absqp"][pj], ctx["qsbp"][pj].bitcast(i16),
                    0x7FFF, None, OP.bitwise_and,
                )
                nc.scalar.activation(
                    ctx["ebp"][pj], ctx["absqp"][pj], AF.Identity,
                    bias=biases[:, 0:1], scale=0.852,
                )

            def hl_pair(ctx, pj):
                nc.vector.tensor_tensor(
                    ctx["hlp"][pj], ctx["qsbp"][pj],
                    ctx["ebp"][pj].bitcast(bf16), OP.mult,
                )

            def h_half(ctx, half, rl, sqb, amdls):
                sc, bt, hlp = ctx["sc"], ctx["bt"], ctx["hlp"]
                hps = [hpool.tile([P, CH], f32, tag="hp",
                                  name=f"hp{sc}_{half}_{n}") for n in range(2)]
                for kc in range(8):
                    mov = hlp[kc // 2][:, (kc % 2) * CH : (kc % 2 + 1) * CH]
                    for n in range(2):
                        n_ = half * 2 + n
                        nc.tensor.matmul(
                            hps[n],
                            invpt[:, (kc * 4 + n_) * P : (kc * 4 + n_ + 1) * P],
                            mov,
                            start=(kc == 0),
                            stop=(kc == 7),
                        )
                for n in range(2):
                    n_ = half * 2 + n
                    # invpt is negated host-side: rl = relu(hp + c0^2*hsup)
                    nc.scalar.activation(
                        rl[:, n_ * CH : (n_ + 1) * CH],
                        hps[n],
                        AF.Relu,
                        bias=hsup[:, n_ : n_ + 1],
                        scale=1.0,
                    )
                hsl = slice(half * 2 * CH, (half + 1) * 2 * CH)
                # sq = c0*sqrt(rl) via exponent bit-hack
                nc.vector.tensor_scalar(
                    sqb[:, hsl], rl.bitcast(i16)[:, hsl], 0.5, CS, OP.mult, OP.add
                )
                for n in range(2):
                    n_ = half * 2 + n
                    dl = spool.tile([P, CH], bf16, tag="dl")
                    nc.vector.tensor_tensor(
                        dl,
                        bt[:, B_MA + n_ * CH : B_MA + (n_ + 1) * CH],
                        sqb.bitcast(bf16)[:, n_ * CH : (n_ + 1) * CH],
                        OP.mult,
                    )
                    amdl = spool.tile([P, CH], bf16, tag="amdl", bufs=8)
                    nc.vector.tensor_tensor(
                        amdl,
                        bt[:, B_MA + (4 + n_) * CH : B_MA + (5 + n_) * CH],
                        dl,
                        OP.subtract,
                    )
                    amdls.append(amdl)

            nxt = None
            for sc in range(NCH):
                cur = nxt if nxt is not None else make_ctx(0)
                if sc == 0:
                    q_pair(cur, 0)
                q_pair(cur, 1)
                hl_pair(cur, 0)
                # previous chunk's residual D-matmuls keep the PE busy while
                # the DVE drains of pairs 0-1 release PSUM banks for 2-3
                state = rps_prev = None
                if pending_resid:
                    state = pending_resid.pop()
                    rps_prev = emit_resid_d(state)
                q_pair(cur, 2)
                hl_pair(cur, 1)
                q_pair(cur, 3)
                hl_pair(cur, 2)
                hl_pair(cur, 3)
                if state is not None:
                    emit_resid_inject(state, rps_prev)
                if pending_square:
                    emit_squares()

                rl = wpool.tile([P, 4 * CH], bf16, tag="rl", name=f"rl{sc}",
                                bufs=2)
                sqb = wpool.tile([P, 4 * CH], i16, tag="sqb", name=f"sqb{sc}",
                                 bufs=2)
                amdls = []
                h_half(cur, 0, rl, sqb, amdls)
                # cross-chunk lookahead: the next chunk's first q pair goes
                # out before this chunk's second H half so the PE never
                # runs dry at the chunk boundary
                nxt = None
                if sc + 1 < NCH:
                    nxt = make_ctx(sc + 1)
                    q_pair(nxt, 0)
                h_half(cur, 1, rl, sqb, amdls)
                pending_resid.append((cur["sc"], cur["bt"], amdls))
            state = pending_resid.pop()
            rps_prev = emit_resid_d(state)
            emit_resid_inject(state, rps_prev)
            emit_squares()
            nc.sync.dma_start(out_d, stats)

    nc.compile()
    return nc


def _blocks(mat, kb, mb):
    # [kb*128, mb*128] -> [128, kb*mb*128], block b = kc*mb + mc
    out = np.empty((P, kb * mb * P), np.float32)
    for kc in range(kb):
        for mc in range(mb):
            b = kc * mb + mc
            out[:, b * P : (b + 1) * P] = mat[
                kc * P : (kc + 1) * P, mc * P : (mc + 1) * P
            ]
    return out


def _to_f8(x):
    a = np.asarray(x, np.float32)
    assert np.all(np.abs(a) < 448.0), "fp8 e4m3 overflow in host prep"
    return a.astype(F8)


def _host_prep(inputs):
    D = np.ascontiguousarray(np.asarray(inputs["D"], np.float32))
    leak = np.asarray(inputs["leak_id"]).reshape(-1).astype(np.int64)
    A0 = np.asarray(inputs["A0"], np.float32)
    inv = np.asarray(inputs["inv"], np.float32)
    M = np.asarray(inputs["M"], np.float32)
    supply = np.asarray(inputs["supply"], np.float32)
    L = np.asarray(inputs["L"], np.float32)
    d = np.asarray(inputs["d"], np.float32)
    C = np.asarray(inputs["C"], np.float32)
    a = float(np.asarray(inputs["a"]))
    Cd = float(np.asarray(inputs["Cd"]))
    W1 = np.asarray(inputs["W1"], np.float32)
    b1 = np.asarray(inputs["b1"], np.float32)
    W2 = np.asarray(inputs["W2"], np.float32)
    b2 = np.asarray(inputs["b2"], np.float32)
    W3 = np.asarray(inputs["W3"], np.float32)
    b3 = np.asarray(inputs["b3"], np.float32)
    base = np.asarray(inputs["base"], np.float32)

    # per-pipe net table (memoized MLP over the 1024 possible leak ids)
    ids = np.arange(N_PIPES, dtype=np.float32)[:, None]
    h = np.tanh(ids @ W1 + b1)
    h = np.tanh(h @ W2 + b2)
    table = base + (h @ W3 + b3)[:, 0]

    perm = np.concatenate([np.arange(0, N_NODES, 2), np.arange(1, N_NODES, 2)])
    Mp = M[perm]
    invp = inv[perm]
    inv_ev = invp[:N_DEM]  # rows of inv at even node indices

    K = 10.667 * C**-1.852 * d**-4.871 * L
    k1 = K ** (1.0 / 1.852)  # fold into q so hL = q'|q'|^0.852

    PM = inv.T @ M                        # [1024p, 1024t]
    c0 = Cd * a * math.sqrt(2.0 * G_ACC)

    # fp8 range calibration for invpt: per-pipe power-of-2 scale s from a
    # host sample of hl = q'|q'|^0.852; s folds out of hl via k1 (exact).
    ns = 2048
    q_s = (D[:ns] @ (inv_ev * k1[None, :])) + (
        (PM * table[None, :]) * k1[:, None]
    ).T[leak[:ns]]
    hl_s = np.abs(q_s) ** 1.852
    rms = np.sqrt(np.mean(hl_s**2, axis=0)) + 1e-30
    s = 2.0 ** np.round(np.log2(rms) - 2.0)
    k1f = (k1 * s ** (-1.0 / 1.852)).astype(np.float32)

    PMn = (PM * table[None, :]) * k1f[:, None]
    A0p = A0[perm]
    AMn = (A0p @ PM) * table[None, :]     # [512n, 1024t]
    A0invT = (A0p @ inv_ev.T).T.copy()    # [256j, 512n]
    A0invT[:, :N_DEM] -= np.eye(N_DEM, dtype=np.float32)  # fold -demand

    invev_l = _to_f8(_blocks(inv_ev * k1f[None, :], 2, 8))
    invpt_l = _to_f8(_blocks(invp.T * (-c0 * c0) * s[:, None], 8, 4))
    a0inv_l = _to_f8(_blocks(A0invT, 2, 4))

    consts = np.zeros((P, 70), np.float32)
    consts[:, 0] = CP
    consts[:, 2:6] = np.ascontiguousarray(
        ((invp @ supply) * (c0 * c0)).reshape(4, P).T
    )
    consts[:, 6:70] = np.eye(P, dtype=np.float32).astype(BF16).view(np.float32)

    PMt = PMn.T.astype(BF16)              # [1024 table, 1024 pipe]
    Mt = Mp.T.astype(BF16)                # [1024 table, 512 node]
    AMt = AMn.T.astype(BF16)              # [1024 table, 512 node]

    bigbs = []
    big8s = []
    for c in range(N_CORES):
        Dc = D[c * SC : (c + 1) * SC]     # [2048, 256]
        DT = np.ascontiguousarray(Dc.T).astype(BF16)  # [256, 2048]
        lc = leak[c * SC : (c + 1) * SC]
        bigb = np.empty((P, NCH * BCH), BF16)
        big8 = np.empty((P, W8_END), F8)
        big8[:, W8_INVEV : W8_INVEV + 16 * P] = invev_l
        big8[:, W8_INVPT:W8_A0INV] = invpt_l
        big8[:, W8_A0INV:W8_END] = a0inv_l
        for sc in range(NCH):
            ssl = slice(sc * CH, (sc + 1) * CH)
            off = sc * BCH
            bigb[:, off : off + CH] = DT[:P, ssl]
            bigb[:, off + CH : off + 2 * CH] = DT[P:, ssl]
            gpm = PMt[lc[ssl]]            # [CH, 1024]
            bigb[:, off + B_PM : off + B_MA] = (
                gpm.reshape(CH, 8, P).transpose(2, 1, 0).reshape(P, 8 * CH)
            )
            gm = Mt[lc[ssl]]              # [CH, 512]
            bigb[:, off + B_MA : off + B_MA + 4 * CH] = (
                gm.reshape(CH, 4, P).transpose(2, 1, 0).reshape(P, 4 * CH)
            )
            gam = AMt[lc[ssl]]
            bigb[:, off + B_MA + 4 * CH : off + BCH] = (
                gam.reshape(CH, 4, P).transpose(2, 1, 0).reshape(P, 4 * CH)
            )
        bigbs.append(np.ascontiguousarray(bigb))
        big8s.append(np.ascontiguousarray(big8))

    return consts, bigbs, big8s


LAST_RESULTS = None


def kernel(**inputs) -> np.ndarray:
    global LAST_RESULTS
    from concourse.bass_utils import run_bass_kernel_spmd

    consts, bigbs, big8s = _host_prep(inputs)

    if "nc" not in _MODULE_CACHE:
        _MODULE_CACHE["nc"] = _build_module()
    nc = _MODULE_CACHE["nc"]

    in_maps = []
    for c in range(N_CORES):
        in_maps.append({"consts": consts, "bigb": bigbs[c], "big8": big8s[c]})

    import os

    res = run_bass_kernel_spmd(
        nc,
        in_maps,
        core_ids=list(range(N_CORES)),
        trace=bool(os.environ.get("BASS_TRACE")),
    )
    LAST_RESULTS = res

    total = 0.0
    for r in res.results:
        total += float(r["out_stats"].astype(np.float64).sum())
    return np.float32(total / (S_TOTAL * N_NODES))


# revision 26
# speedup vs baseline: 1.0246x; 1.0246x over previous
"""Trainium2 Bass kernel for the water-network leak MSE model.

Math (reference):
    net(s)   = base[idx_s] + MLP(idx_s)                    (idx_s in [0,1024))
    y        = net*onehot(idx) @ M^T + demand              demand[:, 2j] = D[:, j]
    q        = y @ inv
    hL       = sign(q) * K * |q|^1.852,  K = 10.667 C^-1.852 d^-4.871 L
    H        = (supply - hL) @ inv^T
    d_leak   = Cd*a*sqrt(2g) * (onehot @ M^T) * sqrt(relu(H))
    out      = mean((q @ A0^T - demand - d_leak)^2)

Device strategy (8 cores, data-parallel over samples, 2048 samples/core,
4 chunks of 512 samples per core). Host folds (validated against the
reference; ~8e-5 rel err total vs the 2e-2 tolerance):
  - PM = inv^T M, AM = A0' PM with the memoized per-pipe net table and the
    Hazen-Williams coefficient k1 = K^(1/1.852) premultiplied; node rows
    permuted even-first; demand folded into the A0inv stationary (-I block);
    invpt negated and scaled by c0^2 so relu(H) needs no extra scaling.
  - leak_id is host-visible, so the per-sample M/PM/AM columns are gathered
    ON THE HOST into dense [feature, sample] inputs - no on-device gather.
  - Matmul stationaries ship as fp8-e4m3 (PE runs them mixed against bf16
    movings at full rate); gather payloads and D stay bf16. invpt carries a
    per-pipe power-of-2 scale s (calibrated from a host sample of hl's rms)
    folded out of hl via k1, exact in the product.
  - Inputs stream in consumption order through one DMA queue (startup is
    HBM-bound, not issue-bound).
Both power chains are exponent bit-hacks on bf16 bit patterns (no ACT
tables, verified exact round-to-nearest semantics on HW):
    |q|^0.852:        e_bits = round(0.852*(q_bits & 0x7fff)) + 2406
    c0*sqrt(relu(H)): s_bits = round(0.5*rl_bits) + 8124
Pipeline per 512-sample chunk: q' is produced in four pipe-chunk PAIRS,
each pair's PSUM drain (+PM column), abs-mask (DVE), 0.852-fma (ACT int16
Identity) and hl multiply (DVE) are separate ops so the H matmuls
(emitted kc-outer, two node chunks at a time) start consuming hl as soon
as the first pair lands. The residual stage of chunk sc runs inside chunk
sc+1 (D-part matmuls between q pairs, identity-injects of AM_col - d_leak
after them, ACT Square+accumulate from PSUM after the fma chain), and the
next chunk's first q pair is emitted before the current chunk's second H
half, so the PE stream never runs dry at chunk boundaries; the HAM clock
gate stays at full rate for the whole kernel.
Each core returns [128, 16] partial sums of squares; the host reduces.
"""

import math

import numpy as np
import ml_dtypes

P = 128
N_CORES = 8
S_TOTAL = 16384
SC = S_TOTAL // N_CORES  # samples per core
CH = 512                 # samples per chunk
NCH = SC // CH           # chunks per core
N_NODES = 512
N_PIPES = 1024
N_DEM = 256
G_ACC = 9.80665

CP = 2406.0   # pow-hack bias:  e_bits = round(0.852*t) + CP
CS = 8124.0   # sqrt-hack bias: s_bits = round(0.5*t) + CS

BF16 = ml_dtypes.bfloat16
F8 = ml_dtypes.float8_e4m3

# bf16 stream layout (per chunk: dt chunk-major, PM gather block, M|AM block)
BCH = CH * 2 + 8 * CH + 8 * CH  # 9216 cols per chunk
B_DT = 0                        # dt offset within a chunk slice
B_PM = 2 * CH                   # gPM offset within a chunk slice
B_MA = 10 * CH                  # gM|gAM offset within a chunk slice
# fp8 stream layout (matmul stationaries only)
W8_INVEV = 0                    # 16 blocks * 128
W8_INVPT = 16 * P               # 32 blocks * 128
W8_A0INV = 48 * P               # 8 blocks * 128
W8_END = 56 * P

_MODULE_CACHE: dict = {}


def _build_module():
    import concourse.bacc as bacc
    import concourse.mybir as mybir
    import concourse.tile as tile

    f32 = mybir.dt.float32
    bf16 = mybir.dt.bfloat16
    i16 = mybir.dt.int16
    fp8 = mybir.dt.float8e4
    AF = mybir.ActivationFunctionType
    OP = mybir.AluOpType

    nc = bacc.Bacc(trn_type="TRN2", target_bir_lowering=False, debug=False)

    consts_d = nc.dram_tensor("consts", [P, 70], f32, kind="ExternalInput").ap()
    big8_d = nc.dram_tensor("big8", [P, W8_END], fp8, kind="ExternalInput").ap()
    bigb_d = nc.dram_tensor("bigb", [P, NCH * BCH], bf16, kind="ExternalInput").ap()
    out_d = nc.dram_tensor("out_stats", [P, NCH * 4], f32, kind="ExternalOutput").ap()

    with tile.TileContext(nc) as tc:
        with (
            tc.tile_pool(name="const", bufs=1) as cpool,
            tc.tile_pool(name="work", bufs=1) as wpool,
            tc.tile_pool(name="small", bufs=2) as spool,
            tc.tile_pool(name="qps", bufs=2, space="PSUM") as qpool,
            tc.tile_pool(name="hps", bufs=2, space="PSUM") as hpool,
            tc.tile_pool(name="rps", bufs=2, space="PSUM") as rpool,
        ):
            # ---- input stream, in consumption order on one queue ----
            consts = cpool.tile_from(consts_d)
            biases = consts[:, 0:2]
            hsup = consts[:, 2:6]
            ident = consts.bitcast(bf16)[:, 12:140]

            invev = cpool.tile([P, 16 * P], fp8, tag="invev")
            nc.sync.dma_start(invev, big8_d[:, W8_INVEV : W8_INVEV + 16 * P])
            btiles = []
            for sc in range(NCH):
                btiles.append(
                    cpool.tile([P, BCH], bf16, tag=f"bt{sc}", name=f"bt{sc}")
                )
            # first chunk: dt + PM pair 0 land first so the q matmuls and
            # the first drain start ~3us earlier; the rest of PM follows
            nc.sync.dma_start(
                btiles[0][:, 0 : B_PM + 2 * CH], bigb_d[:, 0 : B_PM + 2 * CH]
            )
            nc.sync.dma_start(
                btiles[0][:, B_PM + 2 * CH : B_MA],
                bigb_d[:, B_PM + 2 * CH : B_MA],
            )
            wt8 = cpool.tile([P, 40 * P], fp8, tag="wt8")  # invpt | a0inv
            nc.sync.dma_start(wt8, big8_d[:, W8_INVPT:W8_END])
            invpt = wt8[:, 0 : 32 * P]
            a0inv = wt8[:, 32 * P : 40 * P]
            nc.sync.dma_start(btiles[0][:, B_MA:BCH], bigb_d[:, B_MA:BCH])
            for sc in range(1, NCH):
                nc.sync.dma_start(
                    btiles[sc][:, 0:B_MA],
                    bigb_d[:, sc * BCH : sc * BCH + B_MA],
                )
                nc.sync.dma_start(
                    btiles[sc][:, B_MA:BCH],
                    bigb_d[:, sc * BCH + B_MA : (sc + 1) * BCH],
                )
            stats = cpool.tile([P, NCH * 4], f32, tag="stats")

            # residual stage of chunk sc runs during chunk sc+1: the PE
            # matmuls right after chunk sc+1's early q pairs, the ACT
            # squares after chunk sc+1's fma chain
            pending_resid = []
            pending_square = []

            def emit_resid_d(state):
                sc, bt, amdls = state
                rps = []
                for n_ in range(4):
                    rp = rpool.tile([P, CH], f32, tag="rp")
                    nc.tensor.matmul(
                        rp,
                        a0inv[:, (0 * 4 + n_) * P : (0 * 4 + n_ + 1) * P],
                        bt[:, B_DT : B_DT + CH],
                        start=True,
                        stop=False,
                    )
                    nc.tensor.matmul(
                        rp,
                        a0inv[:, (1 * 4 + n_) * P : (1 * 4 + n_ + 1) * P],
                        bt[:, B_DT + CH : B_DT + 2 * CH],
                        start=False,
                        stop=False,
                    )
                    rps.append(rp)
                return rps

            def emit_resid_inject(state, rps):
                sc, bt, amdls = state
                for n_ in range(4):
                    nc.tensor.matmul(rps[n_], ident, amdls[n_], start=False, stop=True)
                pending_square.append((sc, rps))

            def emit_squares():
                sc, rps = pending_square.pop()
                scr = spool.tile([P, CH], bf16, tag="scr")
                for n_ in range(4):
                    nc.scalar.activation(
                        scr,
                        rps[n_],
                        AF.Square,
                        accum_out=stats[:, sc * 4 + n_ : sc * 4 + n_ + 1],
                    )

            def make_ctx(sc):
                ctx = {"sc": sc, "bt": btiles[sc], "qsbp": [], "absqp": [],
                       "ebp": [], "hlp": []}
                for pj in range(4):
                    ctx["qsbp"].append(wpool.tile(
                        [P, 2 * CH], bf16, tag=f"qsbp{pj}",
                        name=f"qsbp{pj}_{sc}", bufs=2))
                    ctx["absqp"].append(wpool.tile(
                        [P, 2 * CH], i16, tag=f"absqp{pj}",
                        name=f"absqp{pj}_{sc}", bufs=2))
                    ctx["ebp"].append(wpool.tile(
                        [P, 2 * CH], i16, tag=f"ebp{pj}",
                        name=f"ebp{pj}_{sc}", bufs=2))
                    ctx["hlp"].append(wpool.tile(
                        [P, 2 * CH], bf16, tag=f"hlp{pj}",
                        name=f"hlp{pj}_{sc}", bufs=2))
                return ctx

            def q_pair(ctx, pj):
                bt = ctx["bt"]
                qp2 = qpool.tile([P, 2 * CH], f32, tag="qp2")
                for h in range(2):
                    pc = pj * 2 + h
                    dst = qp2[:, h * CH : (h + 1) * CH]
                    nc.tensor.matmul(
                        dst,
                        invev[:, (0 * 8 + pc) * P : (0 * 8 + pc + 1) * P],
                        bt[:, B_DT : B_DT + CH],
                        start=True,
                        stop=False,
                    )
                    nc.tensor.matmul(
                        dst,
                        invev[:, (1 * 8 + pc) * P : (1 * 8 + pc + 1) * P],
                        bt[:, B_DT + CH : B_DT + 2 * CH],
                        start=False,
                        stop=True,
                    )
                nc.vector.tensor_tensor(
                    ctx["qsbp"][pj],
                    qp2,
                    bt[:, B_PM + pj * 2 * CH : B_PM + (pj + 1) * 2 * CH],
                    OP.add,
                )
                nc.vector.tensor_scalar(
                    ctx["absqp"][pj], ctx["qsbp"][pj].bitcast(i16),
                    0x7FFF, None, OP.bitwise_and,
                )
                nc.scalar.activation(
                    ctx["ebp"][pj], ctx["absqp"][pj], AF.Identity,
                    bias=biases[:, 0:1], scale=0.852,
                )

            def hl_pair(ctx, pj):
                nc.vector.tensor_tensor(
                    ctx["hlp"][pj], ctx["qsbp"][pj],
                    ctx["ebp"][pj].bitcast(bf16), OP.mult,
                )

            def h_half(ctx, half, rl, sqb, amdls):
                sc, bt, hlp = ctx["sc"], ctx["bt"], ctx["hlp"]
                hps = [hpool.tile([P, CH], f32, tag="hp",
                                  name=f"hp{sc}_{half}_{n}") for n in range(2)]
                for kc in range(8):
                    mov = hlp[kc // 2][:, (kc % 2) * CH : (kc % 2 + 1) * CH]
                    for n in range(2):
                        n_ = half * 2 + n
                        nc.tensor.matmul(
                            hps[n],
                            invpt[:, (kc * 4 + n_) * P : (kc * 4 + n_ + 1) * P],
                            mov,
                            start=(kc == 0),
                            stop=(kc == 7),
                        )
                for n in range(2):
                    n_ = half * 2 + n
                    # invpt is negated host-side: rl = relu(hp + c0^2*hsup)
                    nc.scalar.activation(
                        rl[:, n_ * CH : (n_ + 1) * CH],
                        hps[n],
                        AF.Relu,
                        bias=hsup[:, n_ : n_ + 1],
                        scale=1.0,
                    )
                hsl = slice(half * 2 * CH, (half + 1) * 2 * CH)
                # sq = c0*sqrt(rl) via exponent bit-hack
                nc.vector.tensor_scalar(
                    sqb[:, hsl], rl.bitcast(i16)[:, hsl], 0.5, CS, OP.mult, OP.add
                )
                for n in range(2):
                    n_ = half * 2 + n
                    dl = spool.tile([P, CH], bf16, tag="dl")
                    nc.vector.tensor_tensor(
                        dl,
                        bt[:, B_MA + n_ * CH : B_MA + (n_ + 1) * CH],
                        sqb.bitcast(bf16)[:, n_ * CH : (n_ + 1) * CH],
                        OP.mult,
                    )
                    amdl = spool.tile([P, CH], bf16, tag="amdl", bufs=8)
                    nc.vector.tensor_tensor(
                        amdl,
                        bt[:, B_MA + (4 + n_) * CH : B_MA + (5 + n_) * CH],
                        dl,
                        OP.subtract,
                    )
                    amdls.append(amdl)

            nxt = None
            for sc in range(NCH):
                cur = nxt if nxt is not None else make_ctx(0)
                if sc == 0:
                    q_pair(cur, 0)
                q_pair(cur, 1)
                hl_pair(cur, 0)
                # previous chunk's residual D-matmuls keep the PE busy while
                # the DVE drains of pairs 0-1 release PSUM banks for 2-3
                state = rps_prev = None
                if pending_resid:
                    state = pending_resid.pop()
                    rps_prev = emit_resid_d(state)
                q_pair(cur, 2)
                hl_pair(cur, 1)
                q_pair(cur, 3)
                hl_pair(cur, 2)
                hl_pair(cur, 3)
                if state is not None:
                    emit_resid_inject(state, rps_prev)
                if pending_square:
                    emit_squares()

                rl = wpool.tile([P, 4 * CH], bf16, tag="rl", name=f"rl{sc}",
                                bufs=2)
                sqb = wpool.tile([P, 4 * CH], i16, tag="sqb", name=f"sqb{sc}",
                                 bufs=2)
                amdls = []
                h_half(cur, 0, rl, sqb, amdls)
                # cross-chunk lookahead: the next chunk's first q pair goes
                # out before this chunk's second H half so the PE never
                # runs dry at the chunk boundary
                nxt = None
                if sc + 1 < NCH:
                    nxt = make_ctx(sc + 1)
                    q_pair(nxt, 0)
                h_half(cur, 1, rl, sqb, amdls)
                pending_resid.append((cur["sc"], cur["bt"], amdls))
            state = pending_resid.pop()
            rps_prev = emit_resid_d(state)
            emit_resid_inject(state, rps_prev)
            emit_squares()
            nc.sync.dma_start(out_d, stats)

    nc.compile()
    return nc


def _blocks(mat, kb, mb):
    # [kb*128, mb*128] -> [128, kb*mb*128], block b = kc*mb + mc
    out = np.empty((P, kb * mb * P), np.float32)
    for kc in range(kb):
        for mc in range(mb):
            b = kc * mb + mc
            out[:, b * P : (b + 1) * P] = mat[
                kc * P : (kc + 1) * P, mc * P : (mc + 1) * P
            ]
    return out


def _to_f8(x):
    a = np.asarray(x, np.float32)
    assert np.all(np.abs(a) < 448.0), "fp8 e4m3 overflow in host prep"
    return a.astype(F8)


def _host_prep(inputs):
    D = np.ascontiguousarray(np.asarray(inputs["D"], np.float32))
    leak = np.asarray(inputs["leak_id"]).reshape(-1).astype(np.int64)
    A0 = np.asarray(inputs["A0"], np.float32)
    inv = np.asarray(inputs["inv"], np.float32)
    M = np.asarray(inputs["M"], np.float32)
    supply = np.asarray(inputs["supply"], np.float32)
    L = np.asarray(inputs["L"], np.float32)
    d = np.asarray(inputs["d"], np.float32)
    C = np.asarray(inputs["C"], np.float32)
    a = float(np.asarray(inputs["a"]))
    Cd = float(np.asarray(inputs["Cd"]))
    W1 = np.asarray(inputs["W1"], np.float32)
    b1 = np.asarray(inputs["b1"], np.float32)
    W2 = np.asarray(inputs["W2"], np.float32)
    b2 = np.asarray(inputs["b2"], np.float32)
    W3 = np.asarray(inputs["W3"], np.float32)
    b3 = np.asarray(inputs["b3"], np.float32)
    base = np.asarray(inputs["base"], np.float32)

    # per-pipe net table (memoized MLP over the 1024 possible leak ids)
    ids = np.arange(N_PIPES, dtype=np.float32)[:, None]
    h = np.tanh(ids @ W1 + b1)
    h = np.tanh(h @ W2 + b2)
    table = base + (h @ W3 + b3)[:, 0]

    perm = np.concatenate([np.arange(0, N_NODES, 2), np.arange(1, N_NODES, 2)])
    Mp = M[perm]
    invp = inv[perm]
    inv_ev = invp[:N_DEM]  # rows of inv at even node indices

    K = 10.667 * C**-1.852 * d**-4.871 * L
    k1 = K ** (1.0 / 1.852)  # fold into q so hL = q'|q'|^0.852

    PM = inv.T @ M                        # [1024p, 1024t]
    c0 = Cd * a * math.sqrt(2.0 * G_ACC)

    # fp8 range calibration for invpt: per-pipe power-of-2 scale s from a
    # host sample of hl = q'|q'|^0.852; s folds out of hl via k1 (exact).
    ns = 2048
    q_s = (D[:ns] @ (inv_ev * k1[None, :])) + (
        (PM * table[None, :]) * k1[:, None]
    ).T[leak[:ns]]
    hl_s = np.abs(q_s) ** 1.852
    rms = np.sqrt(np.mean(hl_s**2, axis=0)) + 1e-30
    s = 2.0 ** np.round(np.log2(rms) - 2.0)
    k1f = (k1 * s ** (-1.0 / 1.852)).astype(np.float32)

    PMn = (PM * table[None, :]) * k1f[:, None]
    A0p = A0[perm]
    AMn = (A0p @ PM) * table[None, :]     # [512n, 1024t]
    A0invT = (A0p @ inv_ev.T).T.copy()    # [256j, 512n]
    A0invT[:, :N_DEM] -= np.eye(N_DEM, dtype=np.float32)  # fold -demand

    invev_l = _to_f8(_blocks(inv_ev * k1f[None, :], 2, 8))
    invpt_l = _to_f8(_blocks(invp.T * (-c0 * c0) * s[:, None], 8, 4))
    a0inv_l = _to_f8(_blocks(A0invT, 2, 4))

    consts = np.zeros((P, 70), np.float32)
    consts[:, 0] = CP
    consts[:, 2:6] = np.ascontiguousarray(
        ((invp @ supply) * (c0 * c0)).reshape(4, P).T
    )
    consts[:, 6:70] = np.eye(P, dtype=np.float32).astype(BF16).view(np.float32)

    PMt = PMn.T.astype(BF16)              # [1024 table, 1024 pipe]
    Mt = Mp.T.astype(BF16)                # [1024 table, 512 node]
    AMt = AMn.T.astype(BF16)              # [1024 table, 512 node]

    bigbs = []
    big8s = []
    for c in range(N_CORES):
        Dc = D[c * SC : (c + 1) * SC]     # [2048, 256]
        DT = np.ascontiguousarray(Dc.T).astype(BF16)  # [256, 2048]
        lc = leak[c * SC : (c + 1) * SC]
        bigb = np.empty((P, NCH * BCH), BF16)
        big8 = np.empty((P, W8_END), F8)
        big8[:, W8_INVEV : W8_INVEV + 16 * P] = invev_l
        big8[:, W8_INVPT:W8_A0INV] = invpt_l
        big8[:, W8_A0INV:W8_END] = a0inv_l
        for sc in range(NCH):
            ssl = slice(sc * CH, (sc + 1) * CH)
            off = sc * BCH
            bigb[:, off : off + CH] = DT[:P, ssl]
            bigb[:, off + CH : off + 2 * CH] = DT[P:, ssl]
            gpm = PMt[lc[ssl]]            # [CH, 1024]
            bigb[:, off + B_PM : off + B_MA] = (
                gpm.reshape(CH, 8, P).transpose(2, 1, 0).reshape(P, 8 * CH)
            )
            gm = Mt[lc[ssl]]              # [CH, 512]
            bigb[:, off + B_MA : off + B_MA + 4 * CH] = (
                gm.reshape(CH, 4, P).transpose(2, 1, 0).reshape(P, 4 * CH)
            )
            gam = AMt[lc[ssl]]
            bigb[:, off + B_MA + 4 * CH : off + BCH] = (
                gam.reshape(CH, 4, P).transpose(2, 1, 0).reshape(P, 4 * CH)
            )
        bigbs.append(np.ascontiguousarray(bigb))
        big8s.append(np.ascontiguousarray(big8))

    return consts, bigbs, big8s


LAST_RESULTS = None


def kernel(**inputs) -> np.ndarray:
    global LAST_RESULTS
    from concourse.bass_utils import run_bass_kernel_spmd

    consts, bigbs, big8s = _host_prep(inputs)

    if "nc" not in _MODULE_CACHE:
        _MODULE_CACHE["nc"] = _build_module()
    nc = _MODULE_CACHE["nc"]

    in_maps = []
    for c in range(N_CORES):
        in_maps.append({"consts": consts, "bigb": bigbs[c], "big8": big8s[c]})

    import os

    res = run_bass_kernel_spmd(
        nc,
        in_maps,
        core_ids=list(range(N_CORES)),
        trace=bool(os.environ.get("BASS_TRACE")),
    )
    LAST_RESULTS = res

    total = 0.0
    for r in res.results:
        total += float(r["out_stats"].astype(np.float64).sum())
    return np.float32(total / (S_TOTAL * N_NODES))
